# revision 1
# baseline (speedup 1.0000x reference)
"""EntNet Trainium2 kernel (8-core data-parallel over batch).

Reference computation (shapes: B=64, S=128, L=32, D=100, M=20, V=50000):
  sents = (emb[tokens] * mult).sum(axis=2)            # [B,S,D]
  mem0 = broadcast(keys)                              # [B,M,D]
  per step t: gate = sigmoid(s.mem + s.keys); cand = prelu(mem@Uw.T + keys@Vw.T + s@Ww.T)
              mem = normalize(mem + cand*gate, axis=D)

Kernel strategy per core (8 batches/core, R = 8*20 = 160 (b,m) rows):
  - Embedding gather via gpsimd indirect DMA (2048 rows/instr), reduced
    words->sentences with PE block-ones matmuls straight into D-major
    sents [100, 1024] (t-major columns: col = 8*t + b).
  - Recurrence kept in scale-free form: with U unnormalized and
    rho = 1/||U|| per row, the update
        mem' = normalize(mem + cand * sigmoid(l))
    is exactly
        U' = (1 + exp(-l)) . U + Uw@U + (Vk + Ws_t) * (1/rho)
    which needs no sigmoid and no division; rho' = rsqrt(||U'||^2) is
    computed as Exp(-0.5*Ln(ss)) so the whole loop uses one ACT table set
    (natural_log_exp_and_others: Exp/Ln/Square/Prelu).
"""

import numpy as np

B, S, L, D, M, V = 64, 128, 32, 100, 20, 50000
NCORES = 8
BL = B // NCORES            # 8 batches per core
NS = BL * S                 # 1024 sentences per core
R = BL * M                  # 160 (b, m) rows per core
NTOK = BL * S * L           # 32768 tokens per core
GCH = 16                    # gather chunks (indirect DMA instructions)
KPC = NTOK // (128 * GCH)   # index columns per chunk = 16
RESCALE = 8                 # renormalize U every RESCALE steps (f32 range)

_prog_cache = {}

_ENGINE_SEM = {"PE": "PE_", "DVE": "DVE_", "Activation": "Activation_",
               "Pool": "Pool_", "SP": "SP_"}


def _strip_redundant_self_waits(nc):
    """Legalize sync waits: walrus rejects >1 sync wait on most instruction
    structs. For any instruction carrying several, hoist all but one onto
    preceding single-wait NoOps on the same engine queue (in-order dispatch
    keeps semantics). The instruction keeps its OWN-engine wait if it has one
    (that wait guards an engine-pipelining RAW hazard and must gate execution,
    not just dispatch).
    """
    import concourse.mybir as mybir
    for fn in nc.m.functions:
        for blk in fn.blocks:
            i = 0
            while i < len(blk.instructions):
                inst = blk.instructions[i]
                si = inst.sync_info() if callable(inst.sync_info) else inst.sync_info
                if (si is not None and si.on_wait and len(si.on_wait) > 1
                        and inst.engine is not None):
                    waits = list(si.on_wait)
                    pref = _ENGINE_SEM.get(inst.engine.name)
                    keep_idx = None
                    for j, w in enumerate(waits):
                        if pref and w.ant_name.startswith(pref):
                            keep_idx = j
                            break
                    kept = [waits.pop(keep_idx)] if keep_idx is not None else []
                    noops = []
                    for w in waits:
                        nop = mybir.InstNoOp(
                            name=nc.get_next_instruction_name(), ins=[], outs=[])
                        nop.engine = inst.engine
                        nop.sync_info = mybir.SyncInfo(on_wait=[w], on_update=[])
                        nc.register_instruction(nop, overwrite=True)
                        noops.append(nop)
                    inst.sync_info = mybir.SyncInfo(
                        on_wait=kept, on_update=list(si.on_update))
                    blk.instructions[i:i] = noops
                    i += len(noops)
                i += 1


def _build_program(a_is_one: bool, mult_is_ones: bool, alpha: float,
                   n_steps: int = S, dump: bool = False):
    import concourse.bass as bass
    import concourse.tile as tile
    from concourse import mybir
    from contextlib import ExitStack

    f32 = mybir.dt.float32
    i32 = mybir.dt.int32
    AF = mybir.ActivationFunctionType
    OP = mybir.AluOpType

    nc = bass.Bass(trn_type="TRN2")

    # ---- DRAM I/O ----
    # All f32 constants ride in ONE packed tensor -> one DMA -> one DMA-queue
    # semaphore (walrus LDWEIGHTS has very few sync-wait slots; per-constant
    # DMAs land on different queues and overflow it).
    CW = 625 if not mult_is_ones else 525
    tok_d = nc.dram_tensor("tok", [128, 2 * S], i32, kind="ExternalInput").ap()
    emb_d = nc.dram_tensor("emb", [V, D], f32, kind="ExternalInput").ap()
    consts_d = nc.dram_tensor("consts", [128, CW], f32, kind="ExternalInput").ap()
    out_d = nc.dram_tensor("memT", [D, R], f32, kind="ExternalOutput").ap()
    if dump:
        dsents_d = nc.dram_tensor("d_sents", [D, NS], f32, kind="ExternalOutput").ap()
        dkg_d = nc.dram_tensor("d_kg", [S, R], f32, kind="ExternalOutput").ap()
        dws_d = nc.dram_tensor("d_ws", [D, NS], f32, kind="ExternalOutput").ap()
        dvk_d = nc.dram_tensor("d_vk", [D, M], f32, kind="ExternalOutput").ap()
        du_d = nc.dram_tensor("d_u", [D, R], f32, kind="ExternalOutput").ap()
        drho_d = nc.dram_tensor("d_rho", [1, R], f32, kind="ExternalOutput").ap()
        dl_d = nc.dram_tensor("d_l", [1, R], f32, kind="ExternalOutput").ap()

    f32r = mybir.dt.float32r

    def r(ap):
        return ap.bitcast(f32r)

    def bcast_mid(ap_2d, n_mid):
        # [P, k] -> [P, n_mid, k] with stride-0 middle dim
        return bass.AP(ap_2d.tensor, ap_2d.offset,
                       [list(ap_2d.ap[0]), [0, n_mid], list(ap_2d.ap[1])])

    def bcast_last(ap_2d, n_last):
        # [P, k] -> [P, k, n_last] with stride-0 last dim
        return bass.AP(ap_2d.tensor, ap_2d.offset,
                       [list(ap_2d.ap[0]), list(ap_2d.ap[1]), [0, n_last]])

    with tile.TileContext(nc) as tc, ExitStack() as ctx:
        const = ctx.enter_context(tc.tile_pool(name="const", bufs=1))
        # one buffer per gather chunk: slot reuse would need 2 sync waits on the
        # indirect DMA (WAR on PE readers + WAW on the DMA queue) but walrus
        # allows only one on Pool DMA instructions
        gpool = ctx.enter_context(tc.tile_pool(name="gath", bufs=GCH))
        work = ctx.enter_context(tc.tile_pool(name="work", bufs=2))
        ps_setup = ctx.enter_context(tc.tile_pool(name="ps_setup", bufs=2, space="PSUM"))
        ps_loop = ctx.enter_context(tc.tile_pool(name="ps_loop", bufs=1, space="PSUM"))

        # ---- load constants / weights (single DMA) ----
        # tok rides the Pool (SWDGE) path so the indirect gathers that read it
        # don't need a cross-queue semaphore wait (walrus allows only one).
        tok_sb = const.tile([128, 2 * S], i32)
        nc.gpsimd.dma_start(out=tok_sb[:], in_=tok_d)
        consts = const.tile([128, CW], f32)
        nc.sync.dma_start(out=consts[:], in_=consts_d)
        keysT = consts[0:D, 0:M]
        UwT = consts[0:D, 20:120]
        WwT = consts[0:D, 120:220]
        VwT = consts[0:D, 220:320]
        ident = consts[0:D, 320:420]
        blk = consts[0:128, 420:424]
        onesD = consts[0:D, 424:425]
        ones1 = consts[0:1, 425:525]
        if not mult_is_ones:
            multT = consts[0:128, 525:625]

        # ---- Vk = Vw @ keys^T (early; only needs weights) ----
        ps_vk = ps_setup.tile([D, M], f32, tag="pssent", bufs=3, name="ps_vk")
        nc.tensor.matmul(out=ps_vk[:], lhsT=VwT[:], rhs=keysT[:],
                         start=True, stop=True)
        Vk = const.tile([D, M], f32)
        nc.vector.tensor_copy(out=Vk[:], in_=ps_vk[:])

        # ---- gather + reduce to sents [D, NS] (t-major cols: 8t+b) ----
        # One indirect DMA per 128 tokens (the only idx form the HW DGE
        # unrolls correctly: one index per partition). Work proceeds in
        # blocks of GBLK gathers (= 64 sentence cols = 8 recurrence steps)
        # so the recurrence can overlap the Pool-bound gather stream.
        sents_b = [const.tile([D, 64], f32, name=f"sents_b{w}")
                   for w in range(16)]
        Ws_b = [const.tile([D, 64], f32, name=f"ws_b{w}") for w in range(16)]
        GBLK = 16

        def emit_block(w):
            ps_blk = ps_setup.tile([D, 4 * GBLK], f32, tag="pssent", bufs=3,
                                   name=f"ps_blk{w}")
            for gi in range(GBLK):
                gidx = w * GBLK + gi
                g = gpool.tile([128, D], f32, tag="g", name=f"g{gidx}")
                nc.gpsimd.indirect_dma_start(
                    out=g[:],
                    out_offset=None,
                    in_=emb_d,
                    in_offset=bass.IndirectOffsetOnAxis(
                        ap=tok_sb[:, gidx:gidx + 1], axis=0),
                )
                gc = g[:]
                if not mult_is_ones:
                    gm = gpool.tile([128, D], f32, tag="gm", name=f"gm{gidx}")
                    nc.vector.tensor_tensor(out=gm[:], in0=gc, in1=multT[:],
                                            op=OP.mult)
                    gc = gm[:]
                nc.tensor.matmul(out=ps_blk[:, 4 * gi:4 * gi + 4],
                                 lhsT=gc, rhs=blk[:], start=True, stop=True)
            nc.vector.tensor_copy(out=sents_b[w][:], in_=ps_blk[:])
            ps_ws = ps_setup.tile([D, 4 * GBLK], f32, tag="pssent", bufs=3,
                                  name=f"ps_ws{w}")
            nc.tensor.matmul(out=ps_ws[:], lhsT=WwT[:], rhs=sents_b[w][:],
                             start=True, stop=True)
            nc.vector.tensor_copy(out=Ws_b[w][:], in_=ps_ws[:])

        emit_block(0)
        emit_block(1)

        # ---- initial state ----
        U = work.tile([D, R], f32, tag="U")
        nc.vector.tensor_copy(out=U[:].rearrange("d (b m) -> d b m", m=M),
                              in_=bcast_mid(keysT[:], BL))
        vkwsn = work.tile([D, R], f32, tag="vkwsn")
        nc.vector.tensor_tensor(
            out=vkwsn[:].rearrange("d (b m) -> d b m", m=M),
            in0=bcast_mid(Vk[:], BL),
            in1=bcast_last(Ws_b[0][:, 0:BL], M),
            op=OP.add)
        rho = None

        if dump:
            nc.sync.dma_start(out=dsents_d, in_=sents[:])
            nc.sync.dma_start(out=dws_d, in_=Ws[:])
            nc.sync.dma_start(out=dvk_d, in_=Vk[:])

        # ---- recurrence ----
        keysN = None
        for t in range(n_steps):
            if t % 8 == 0 and t // 8 + 2 < 16:
                emit_block(t // 8 + 2)
            vkwsn_flat = vkwsn[:] if hasattr(vkwsn, 'tensor') and vkwsn.ndim == 2 else vkwsn
            # cand (n-scaled): candf = Uw@U + vkwsn, both ready early (off the
            # gate chain)
            psA = ps_loop.tile([D, R], f32, tag="psA")
            nc.tensor.matmul(out=psA[:], lhsT=UwT[:], rhs=U[:],
                             start=True, stop=True)
            candf = work.tile([D, R], f32, tag="candf")
            nc.vector.tensor_tensor(out=candf[:], in0=psA[:], in1=vkwsn_flat,
                                    op=OP.add)

            # gate logits, key-gate folded in and split:
            #   l = rho * [1^T(ks_t) + 1^T(U . s_t)],  ks_t = (n*keys) . s_t
            if t % RESCALE == 0:
                ks = work.tile([D, BL, M], f32, tag="ks", name=f"ks0_{t}")
                nc.vector.tensor_tensor(
                    out=ks[:], in0=bcast_mid(keysT, BL),
                    in1=bcast_last(sents_b[t // 8][:, BL * (t % 8):BL * (t % 8 + 1)], M),
                    op=OP.mult)
            psmg = ps_loop.tile([1, R], f32, tag="psmg")
            nc.tensor.matmul(out=psmg[:], lhsT=onesD[:],
                             rhs=ks[:].rearrange("d b m -> d (b m)"),
                             start=True, stop=False)
            mgt = work.tile([D, BL, M], f32, tag="mgt")
            nc.vector.tensor_tensor(
                out=mgt[:],
                in0=U[:].rearrange("d (b m) -> d b m", m=M),
                in1=bcast_last(sents_b[t // 8][:, BL * (t % 8):BL * (t % 8 + 1)], M),
                op=OP.mult)
            nc.tensor.matmul(out=psmg[:], lhsT=onesD[:],
                             rhs=mgt[:].rearrange("d b m -> d (b m)"),
                             start=False, stop=True)
            if t % RESCALE == 0:
                l_ap = psmg[:]
            else:
                l_sb = work.tile([1, R], f32, tag="l")
                nc.vector.tensor_tensor(out=l_sb[:], in0=psmg[:], in1=rho[:],
                                        op=OP.mult)
                l_ap = l_sb[:]
            e_sb = work.tile([1, R], f32, tag="e")
            nc.scalar.activation(out=e_sb[:], in_=l_ap, func=AF.Exp,
                                 scale=-1.0)

            # U' = (1 + e) . U + cand
            psbce = ps_loop.tile([D, R], f32, tag="psbce")
            nc.tensor.matmul(out=psbce[:], lhsT=ones1[:], rhs=e_sb[:],
                             start=True, stop=True)
            V_sb = work.tile([D, R], f32, tag="V")
            nc.vector.scalar_tensor_tensor(out=V_sb[:], in0=psbce[:],
                                           scalar=1.0, in1=U[:],
                                           op0=OP.add, op1=OP.mult)
            U2 = work.tile([D, R], f32, tag="U")
            if a_is_one:
                nc.vector.tensor_tensor(out=U2[:], in0=candf[:], in1=V_sb[:],
                                        op=OP.add)
            elif False:
                candn = work.tile([D, R], f32, tag="candn")
                nc.scalar.activation(out=candn[:], in_=psA[:], func=AF.Prelu,
                                     alpha=float(alpha))
                nc.vector.tensor_tensor(out=U2[:], in0=candn[:], in1=V_sb[:],
                                        op=OP.add)
            U = U2

            # norms: rho' = exp(-0.5 ln ss), n' = ss * rho'
            sq = work.tile([D, R], f32, tag="sq")
            nc.scalar.activation(out=sq[:], in_=U[:], func=AF.Square)
            psss = ps_loop.tile([1, R], f32, tag="psss")
            nc.tensor.matmul(out=psss[:], lhsT=onesD[:], rhs=sq[:],
                             start=True, stop=True)
            lnss = work.tile([1, R], f32, tag="lnss")
            nc.scalar.activation(out=lnss[:], in_=psss[:], func=AF.Ln)
            rho2 = work.tile([1, R], f32, tag="rho")
            nc.scalar.activation(out=rho2[:], in_=lnss[:], func=AF.Exp,
                                 scale=-0.5)
            rho = rho2

            if dump and t == n_steps - 1:
                nc.sync.dma_start(out=du_d, in_=U[:])
                nc.sync.dma_start(out=drho_d, in_=rho[:])

            rescale_now = ((t + 1) % RESCALE == 0)
            if rescale_now:
                # exact renormalization: U *= bc(rho); afterwards rho = n = 1
                psbcr = ps_loop.tile([D, R], f32, tag="psbcn", name="psbcr_t")
                nc.tensor.matmul(out=psbcr[:], lhsT=ones1[:], rhs=rho[:],
                                 start=True, stop=True)
                U3 = work.tile([D, R], f32, tag="U")
                nc.vector.tensor_tensor(out=U3[:], in0=psbcr[:], in1=U[:],
                                        op=OP.mult)
                U = U3

            if t < S - 1:
                vw = work.tile([D, BL, M], f32, tag="vw")
                nc.vector.tensor_tensor(
                    out=vw[:],
                    in0=bcast_mid(Vk[:], BL),
                    in1=bcast_last(Ws_b[(t + 1) // 8][:, BL * ((t + 1) % 8):BL * ((t + 1) % 8 + 1)], M),
                    op=OP.add)
                if rescale_now:
                    vkwsn = vw[:].rearrange("d b m -> d (b m)")  # n = 1
                else:
                    n_sb = work.tile([1, R], f32, tag="n")
                    nc.vector.tensor_tensor(out=n_sb[:], in0=psss[:], in1=rho[:],
                                            op=OP.mult)
                    psbcn = ps_loop.tile([D, R], f32, tag="psbcn")
                    nc.tensor.matmul(out=psbcn[:], lhsT=ones1[:], rhs=n_sb[:],
                                     start=True, stop=True)
                    vkwsn2 = work.tile([D, R], f32, tag="vkwsn")
                    nc.vector.tensor_tensor(out=vkwsn2[:], in0=psbcn[:],
                                            in1=vw[:].rearrange("d b m -> d (b m)"),
                                            op=OP.mult)
                    vkwsn = vkwsn2
                    keysN = work.tile([D, BL, M], f32, tag="keysN")
                    nc.vector.tensor_tensor(
                        out=keysN[:],
                        in0=bass.AP(psbcn.tensor, psbcn.offset,
                                    [list(psbcn.ap[0]), [M, BL], [1, M]]),
                        in1=bcast_mid(keysT, BL), op=OP.mult)
                    tn = t + 1
                    ks = work.tile([D, BL, M], f32, tag="ks", name=f"ks_{tn}")
                    nc.vector.tensor_tensor(
                        out=ks[:], in0=keysN[:],
                        in1=bcast_last(sents_b[tn // 8][:, BL * (tn % 8):BL * (tn % 8 + 1)], M),
                        op=OP.mult)

        # ---- output: memT = U * bc(rho) (U already unit if last step rescaled) ----
        if n_steps % RESCALE == 0:
            nc.sync.dma_start(out=out_d, in_=U[:])
        else:
            psbcr = ps_loop.tile([D, R], f32, tag="psbcn")
            nc.tensor.matmul(out=psbcr[:], lhsT=ones1[:], rhs=rho[:],
                             start=True, stop=True)
            memT = work.tile([D, R], f32, tag="memT")
            nc.vector.tensor_tensor(out=memT[:], in0=psbcr[:], in1=U[:],
                                    op=OP.mult)
            nc.sync.dma_start(out=out_d, in_=memT[:])

    _strip_redundant_self_waits(nc)
    return nc


def _stage_inputs(tokens, emb, keys, mult, Uw, Vw, Ww, prelu_a):
    """Host-side sharding/layout prep. Returns (in_maps, flags)."""
    tokens = np.asarray(tokens)
    emb = np.ascontiguousarray(np.asarray(emb, dtype=np.float32))
    keys = np.asarray(keys, dtype=np.float32)
    mult = np.asarray(mult, dtype=np.float32)
    a = float(np.asarray(prelu_a).reshape(-1)[0])
    a_is_one = (a == 1.0)
    mult_is_ones = bool(np.all(mult == 1.0))

    CW = 625 if not mult_is_ones else 525
    consts = np.zeros((128, CW), np.float32)
    consts[0:D, 0:M] = keys.T
    consts[0:D, 20:120] = np.asarray(Uw, np.float32).T        # lhsT for Uw@mem
    consts[0:D, 120:220] = np.asarray(Ww, np.float32).T
    consts[0:D, 220:320] = np.asarray(Vw, np.float32).T
    consts[0:D, 320:420] = np.eye(D, dtype=np.float32)
    consts[0:128, 420:424] = np.kron(np.eye(4, dtype=np.float32),
                                     np.ones((32, 1), np.float32))
    consts[0:D, 424:425] = 1.0                                # onesD
    consts[0:1, 425:525] = 1.0                                # ones1
    if not mult_is_ones:
        consts[0:128, 525:625] = np.tile(mult, (4, 1))

    in_maps = []
    for c in range(NCORES):
        tc_ = tokens[c * BL:(c + 1) * BL]                     # [8, S, L]
        # sentence-major rows with t-major sentence order: row j = 8t+b
        tokflat = np.ascontiguousarray(tc_.transpose(1, 0, 2)).reshape(NS, L)
        # tok_staged[p, col] = token of sentence 4*col + p//32, word p%32
        tok_staged = np.ascontiguousarray(
            tokflat.reshape(2 * S, 4, L).transpose(1, 2, 0)).reshape(128, 2 * S)
        in_maps.append({"tok": np.ascontiguousarray(tok_staged, np.int32),
                        "emb": emb, "consts": consts})
    return in_maps, a_is_one, mult_is_ones, a


def kernel(tokens, emb, keys, mult, Uw, Vw, Ww, prelu_a, _trace=False):
    from concourse.bass_utils import run_bass_kernel_spmd

    in_maps, a_is_one, mult_is_ones, a = _stage_inputs(
        tokens, emb, keys, mult, Uw, Vw, Ww, prelu_a)

    key = (a_is_one, mult_is_ones, a)
    if key not in _prog_cache:
        _prog_cache[key] = _build_program(a_is_one, mult_is_ones, a)
    nc = _prog_cache[key]

    res = run_bass_kernel_spmd(nc, in_maps, list(range(NCORES)), trace=_trace)
    out = np.empty((B, M, D), dtype=np.float32)
    for c in range(NCORES):
        memT = res.results[c]["memT"]                          # [D, R]
        out[c * BL:(c + 1) * BL] = memT.reshape(D, BL, M).transpose(1, 2, 0)
    kernel._last_results = res
    return out



# revision 19
# speedup vs baseline: 1.2791x; 1.2791x over previous
"""EntNet Trainium2 kernel (8-core data-parallel over batch), v2.

Reference computation (B=64, S=128, L=32, D=100, M=20, V=50000):
  sents = (emb[tokens] * mult).sum(axis=2)            # [B,S,D]
  mem0 = broadcast(keys)                              # [B,M,D]
  per step t: gate = sigmoid(s.mem + s.keys); cand = prelu(mem@Uw.T + keys@Vw.T + s@Ww.T)
              mem = normalize(mem + cand*gate, axis=D)

Kernel strategy per core (8 batches/core, R = 8*20 = 160 (b,m) rows, layout
[D, R] with D on partitions):
  - Embedding gather via gpsimd indirect DMA (fat form: 16 index columns per
    instruction = 2048 rows, amortizing the ~1us SWDGE fixed cost), reduced
    words->sentences with PE block-ones matmuls into D-major sents [100, 1024]
    (t-major columns: col = 8*t + b).
  - Scale-free recurrence: mem = rho*U with U unnormalized, rho = 1/||U||,
    n = 1/rho. Per step:
        l   = rho*(s.U) + s.keys
        e   = exp(-l)                  (1/gate = 1+e)
        U'  = (1+e)*U + candf,  candf = Uw@U + (Vk + Ws_t)*n
    The two gate reductions ride ONE [100,320] colsum matmul (U*s | keys*s
    padded side by side). ||U'||^2 is NOT recomputed from U'; it is expanded as
        ss' = (1+e)^2*ss + 2(1+e)*p1 + p2,   p1 = U.candf, p2 = ||candf||^2
    where p1/p2 colsums are ready early (one [1,320] matmul), so the norm
    chain (ln/exp for rho'=rsqrt(ss')) starts at e_t instead of U'_t --
    shortening the serial cycle. Exact renormalization every RESCALE steps
    bounds fp32 growth and resets ss/rho/n to exactly 1.
  - All recurrence matmuls run as float32r with the moving dim padded to >=256
    (1 cycle/row instead of fp32's 4).
  - Elementwise work is split across DVE / GpSimd / ACT to balance engines.
"""

import os

import numpy as np

B, S, L, D, M, V = 64, 128, 32, 100, 20, 50000
NCORES = 8
BL = B // NCORES            # 8 batches per core
NS = BL * S                 # 1024 sentences per core
R = BL * M                  # 160 (b, m) rows per core
NTOK = BL * S * L           # 32768 tokens per core
NGATH = NTOK // 128         # 256 gather index columns
GFAT = 16                   # index columns per indirect DMA instruction
NBLK = 16                   # sentence blocks (64 sent cols = 8 steps each)
RESCALE = 8                 # renormalize U every RESCALE steps
RP = 256                    # padded moving dim for f32r matmuls

_prog_cache = {}

_ENGINE_SEM = {"PE": "PE_", "DVE": "DVE_", "Activation": "Activation_",
               "Pool": "Pool_", "SP": "SP_"}


def _strip_redundant_self_waits(nc):
    """Legalize sync waits: walrus rejects >1 sync wait on most instruction
    structs. For any instruction carrying several, hoist all but one onto
    preceding single-wait NoOps on the same engine queue (in-order dispatch
    keeps semantics). The instruction keeps its OWN-engine wait if it has one
    (that wait guards an engine-pipelining RAW hazard and must gate execution,
    not just dispatch).
    """
    import concourse.mybir as mybir
    for fn in nc.m.functions:
        for blk in fn.blocks:
            i = 0
            while i < len(blk.instructions):
                inst = blk.instructions[i]
                si = inst.sync_info() if callable(inst.sync_info) else inst.sync_info
                if (si is not None and si.on_wait and len(si.on_wait) > 1
                        and inst.engine is not None):
                    waits = list(si.on_wait)
                    pref = _ENGINE_SEM.get(inst.engine.name)
                    keep_idx = None
                    for j, w in enumerate(waits):
                        if pref and w.ant_name.startswith(pref):
                            keep_idx = j
                            break
                    kept = [waits.pop(keep_idx)] if keep_idx is not None else []
                    noops = []
                    for w in waits:
                        nop = mybir.InstNoOp(
                            name=nc.get_next_instruction_name(), ins=[], outs=[])
                        nop.engine = inst.engine
                        nop.sync_info = mybir.SyncInfo(on_wait=[w], on_update=[])
                        nc.register_instruction(nop, overwrite=True)
                        noops.append(nop)
                    inst.sync_info = mybir.SyncInfo(
                        on_wait=kept, on_update=list(si.on_update))
                    blk.instructions[i:i] = noops
                    i += len(noops)
                i += 1


def _build_program(a_is_one: bool, mult_is_ones: bool, alpha: float,
                   n_steps: int = S, fat_gather: bool = True,
                   gpsimd_tt: bool = True):
    import concourse.bass as bass
    import concourse.tile as tile
    from concourse import mybir
    from contextlib import ExitStack

    f32 = mybir.dt.float32
    f32r = mybir.dt.float32r
    i32 = mybir.dt.int32
    AF = mybir.ActivationFunctionType
    OP = mybir.AluOpType

    nc = bass.Bass(trn_type="TRN2")

    CW = 686 if not mult_is_ones else 586
    tok_d = nc.dram_tensor("tok", [128, NGATH], i32, kind="ExternalInput").ap()
    emb_d = nc.dram_tensor("emb", [V, D], f32, kind="ExternalInput").ap()
    consts_d = nc.dram_tensor("consts", [128, CW], f32r, kind="ExternalInput").ap()
    out_d = nc.dram_tensor("memT", [D, R], f32, kind="ExternalOutput").ap()

    def r(ap):
        return ap.bitcast(f32r)

    def c(ap):
        # plain-f32 view of an f32r tile for non-matmul readers
        return ap.bitcast(f32)

    def bcast_mid(ap_2d, n_mid):
        # [P, k] -> [P, n_mid, k] with stride-0 middle dim
        return bass.AP(ap_2d.tensor, ap_2d.offset,
                       [list(ap_2d.ap[0]), [0, n_mid], list(ap_2d.ap[1])])

    def bcast_last(ap_2d, n_last):
        # [P, k] -> [P, k, n_last] with stride-0 last dim
        return bass.AP(ap_2d.tensor, ap_2d.offset,
                       [list(ap_2d.ap[0]), list(ap_2d.ap[1]), [0, n_last]])

    with tile.TileContext(nc) as tc, ExitStack() as ctx:
        const = ctx.enter_context(tc.tile_pool(name="const", bufs=1))
        gpool = ctx.enter_context(tc.tile_pool(
            name="gath", bufs=(4 if fat_gather else 16)))
        work = ctx.enter_context(tc.tile_pool(name="work", bufs=2))
        ps_setup = ctx.enter_context(tc.tile_pool(name="ps_setup", bufs=2, space="PSUM"))
        ps_loop = ctx.enter_context(tc.tile_pool(name="ps_loop", bufs=1, space="PSUM"))

        # ---- load constants / weights ----
        # tok rides the Pool (SWDGE) path so the indirect gathers that read it
        # don't need a cross-queue semaphore wait.
        tok_sb = const.tile([128, NGATH], i32)
        nc.gpsimd.dma_start(out=tok_sb[:], in_=tok_d)
        consts = const.tile([128, CW], f32r)
        nc.sync.dma_start(out=consts[:], in_=consts_d)
        keysT = consts[0:D, 0:M]
        UwT = consts[0:D, 20:120]
        WwT = consts[0:D, 120:220]
        VwT = consts[0:D, 220:320]
        blk = consts[0:128, 320:324]
        onesD = consts[0:D, 324:325]
        ones1 = consts[0:1, 325:425]
        onesR = consts[0:1, 425:585]
        if not mult_is_ones:
            multT = consts[0:128, 585:685]

        # ---- Vk = Vw @ keys^T ----
        ps_vk = ps_setup.tile([D, M], f32, tag="pssent", bufs=3, name="ps_vk")
        nc.tensor.matmul(out=ps_vk[:], lhsT=c(VwT[:]), rhs=c(keysT[:]),
                         start=True, stop=True)
        Vk = const.tile([D, M], f32)
        nc.vector.tensor_copy(out=Vk[:], in_=ps_vk[:])

        # ---- gather + reduce to sents [D, NS] (t-major cols: 8t+b) ----
        sents_b = [const.tile([D, 64], f32, name=f"sents_b{w}")
                   for w in range(NBLK)]
        Ws_b = [const.tile([D, 64], f32, name=f"ws_b{w}") for w in range(NBLK)]

        gtiles = {}
        ps_blks = {}

        def emit_gather(w):
            if fat_gather:
                g = gpool.tile([128, GFAT, D], f32, tag="g", name=f"g{w}")
                nc.gpsimd.indirect_dma_start(
                    out=g[:],
                    out_offset=None,
                    in_=emb_d,
                    in_offset=bass.IndirectOffsetOnAxis(
                        ap=tok_sb[:, w * GFAT:(w + 1) * GFAT], axis=0),
                )
                gtiles[w] = [g[:, gi, :] for gi in range(GFAT)]
            else:
                gs = []
                for gi in range(GFAT):
                    gidx = w * GFAT + gi
                    g = gpool.tile([128, D], f32, tag="g", name=f"g{gidx}")
                    nc.gpsimd.indirect_dma_start(
                        out=g[:],
                        out_offset=None,
                        in_=emb_d,
                        in_offset=bass.IndirectOffsetOnAxis(
                            ap=tok_sb[:, gidx:gidx + 1], axis=0),
                    )
                    gs.append(g[:])
                gtiles[w] = gs

        def emit_reduce(w, quarter):
            if quarter == 0:
                ps_blks[w] = ps_setup.tile([D, 64], f32, tag="pssent", bufs=3,
                                           name=f"ps_blk{w}")
            ps_blk = ps_blks[w]
            for gi in range(4 * quarter, 4 * quarter + 4):
                gc = gtiles[w][gi]
                if not mult_is_ones:
                    gm = gpool.tile([128, D], f32, tag="gm", name=f"gm{w}_{gi}")
                    nc.vector.tensor_tensor(out=gm[:], in0=gc, in1=multT[:],
                                            op=OP.mult)
                    gc = gm[:]
                nc.tensor.matmul(out=ps_blk[:, 4 * gi:4 * gi + 4],
                                 lhsT=gc, rhs=c(blk[:]), start=True, stop=True)

        def emit_finish(w):
            nc.vector.tensor_copy(out=sents_b[w][:], in_=ps_blks[w][:])
            ps_ws = ps_setup.tile([D, 64], f32, tag="pssent", bufs=3,
                                  name=f"ps_ws{w}")
            nc.tensor.matmul(out=ps_ws[:], lhsT=c(WwT[:]), rhs=sents_b[w][:],
                             start=True, stop=True)
            nc.vector.tensor_copy(out=Ws_b[w][:], in_=ps_ws[:])

        def emit_block(w):
            emit_gather(w)
            for q in range(4):
                emit_reduce(w, q)
            emit_finish(w)

        emit_block(0)
        emit_block(1)

        # ---- fixed state tiles (manual rotation; pads memset once) ----
        zcolD = consts[0:D, CW - 1:CW]
        zcol1 = consts[0:1, CW - 1:CW]
        Utiles = [const.tile([D, RP], f32r, name=f"Ust{i}") for i in range(3)]
        for i in range(3):
            nc.vector.tensor_copy(out=Utiles[i][:, R:RP],
                                  in_=bcast_last(zcolD, RP - R))
        mgtP = [const.tile([D, 2 * R], f32r, name=f"mgtP{i}") for i in range(2)]
        pairT = [const.tile([D, 2 * R], f32r, name=f"pairT{i}") for i in range(2)]
        e_t = [const.tile([1, RP], f32r, name=f"e_t{i}") for i in range(2)]
        n_t = [const.tile([1, RP], f32r, name=f"n_t{i}") for i in range(2)]
        rho_t = [const.tile([1, RP], f32r, name=f"rho_t{i}") for i in range(2)]
        for i in range(2):
            nc.vector.tensor_copy(out=e_t[i][:, R:RP],
                                  in_=bcast_last(zcol1, RP - R))
            nc.vector.tensor_copy(out=n_t[i][:, R:RP],
                                  in_=bcast_last(zcol1, RP - R))
            nc.vector.tensor_copy(out=rho_t[i][:, R:RP],
                                  in_=bcast_last(zcol1, RP - R))

        # U0 = broadcast(keys) over batches
        ucur = 0
        nc.vector.tensor_copy(
            out=Utiles[0][:, 0:R].rearrange("d (b m) -> d b m", m=M),
            in_=bcast_mid(c(keysT), BL))

        # ss0 = ||keys||^2 per (b, m) row
        sqk = work.tile([D, BL, M], f32, tag="sq0")
        nc.vector.tensor_tensor(out=sqk[:], in0=bcast_mid(c(keysT), BL),
                                in1=bcast_mid(c(keysT), BL), op=OP.mult)
        ps_ss0 = ps_loop.tile([1, R], f32, tag="p1p2", name="ps_ss0")
        nc.tensor.matmul(out=ps_ss0[:], lhsT=c(onesD[:]),
                         rhs=sqk[:].rearrange("d b m -> d (b m)"),
                         start=True, stop=True)
        ss0 = const.tile([1, R], f32)
        nc.vector.tensor_copy(out=ss0[:], in_=ps_ss0[:])

        # state handles
        ss = ss0[:]          # [1,R] ap; ||U||^2 per row
        rho = None           # None => rho == 1 (mem == U)
        n_pad = None         # [1,RP] padded view of n tile; None => n == 1
        VE, AC = nc.vector, nc.scalar
        GE = nc.gpsimd if gpsimd_tt else nc.vector

        for t in range(n_steps):
            w, c8 = t // 8, t % 8
            gath_step = (c8 == 2 and w + 2 < NBLK)
            if gath_step:
                emit_gather(w + 2)
            if c8 >= 4 and w + 2 < NBLK:
                emit_reduce(w + 2, c8 - 4)
                if c8 == 7:
                    emit_finish(w + 2)
            # on gather-emission steps the Pool queue is busy generating DMA
            # descriptors; route that step's gpsimd work to DVE instead
            G = VE if gath_step else GE

            Ucur = Utiles[ucur]
            Unext = Utiles[(ucur + 1) % 3]
            mg = mgtP[t % 2]
            pr = pairT[t % 2]
            ecur = e_t[t % 2]
            ncur = n_t[t % 2]
            rcur = rho_t[t % 2]
            s_sl = sents_b[w][:, BL * c8:BL * (c8 + 1)]      # [100, 8]
            ws_sl = Ws_b[w][:, BL * c8:BL * (c8 + 1)]

            # --- early: candidate + gate reductions (PE) ---
            psA = ps_loop.tile([D, RP], f32, tag="psA")
            nc.tensor.matmul(out=psA[:], lhsT=r(UwT[:]), rhs=r(Ucur[:]),
                             start=True, stop=True)
            if n_pad is not None:
                psbcn = ps_loop.tile([D, RP], f32, tag="psbcn")
                nc.tensor.matmul(out=psbcn[:], lhsT=r(ones1[:]),
                                 rhs=r(n_pad), start=True, stop=True)
            # ks -> pad half of mgt tile: keys*s (gate key part)
            G.tensor_tensor(
                out=mg[:, R:2 * R].rearrange("d (b m) -> d b m", m=M),
                in0=bcast_mid(c(keysT), BL), in1=bcast_last(s_sl, M),
                op=OP.mult)
            # vw = Vk + Ws_t
            vw = work.tile([D, BL, M], f32, tag="vw")
            G.tensor_tensor(out=vw[:], in0=bcast_mid(Vk[:], BL),
                            in1=bcast_last(ws_sl, M), op=OP.add)
            # mgt -> first half: U*s (gate mem part)
            VE.tensor_tensor(
                out=mg[:, 0:R].rearrange("d (b m) -> d b m", m=M),
                in0=c(Ucur[:, 0:R]).rearrange("d (b m) -> d b m", m=M),
                in1=bcast_last(s_sl, M), op=OP.mult)
            psmg = ps_loop.tile([1, 2 * R], f32, tag="psmg")
            nc.tensor.matmul(out=psmg[:], lhsT=r(onesD[:]), rhs=r(mg[:]),
                             start=True, stop=True)

            # candf = Uw@U + vw*n
            vw_flat = vw[:].rearrange("d b m -> d (b m)")
            candf = work.tile([D, R], f32, tag="candf")
            if n_pad is not None:
                c1 = work.tile([D, R], f32, tag="c1")
                VE.tensor_tensor(out=c1[:], in0=vw_flat, in1=psbcn[:, 0:R],
                                 op=OP.mult)
                VE.tensor_tensor(out=candf[:], in0=psA[:, 0:R], in1=c1[:],
                                 op=OP.add)
            else:
                VE.tensor_tensor(out=candf[:], in0=psA[:, 0:R], in1=vw_flat,
                                 op=OP.add)
            cand = candf
            if not a_is_one:
                candp = work.tile([D, R], f32, tag="candp")
                AC.activation(out=candp[:], in_=candf[:], func=AF.Prelu,
                              alpha=float(alpha))
                cand = candp
            # pair tile: [U*cand | cand^2] for the ss' expansion
            G.tensor_tensor(out=pr[:, 0:R], in0=c(Ucur[:, 0:R]), in1=cand[:],
                            op=OP.mult)

            # --- gate: l = rho*(s.U) + s.keys ; e = exp(-l) ---
            if rho is not None:
                l1 = work.tile([1, R], f32, tag="l1")
                VE.tensor_tensor(out=l1[:], in0=psmg[:, 0:R], in1=rho,
                                 op=OP.mult)
                l_sb = work.tile([1, R], f32, tag="l")
                VE.tensor_tensor(out=l_sb[:], in0=l1[:], in1=psmg[:, R:2 * R],
                                 op=OP.add)
            else:
                l1 = work.tile([1, R], f32, tag="l1")
                VE.tensor_copy(out=l1[:], in_=psmg[:, 0:R])
                l_sb = work.tile([1, R], f32, tag="l")
                VE.tensor_tensor(out=l_sb[:], in0=psmg[:, R:2 * R],
                                 in1=l1[:], op=OP.add)
            AC.activation(out=ecur[:, 0:R], in_=l_sb[:], func=AF.Exp,
                          scale=-1.0)
            AC.activation(out=pr[:, R:2 * R], in_=cand[:], func=AF.Square)

            psbce = ps_loop.tile([D, RP], f32, tag="psbce")
            nc.tensor.matmul(out=psbce[:], lhsT=r(ones1[:]), rhs=r(ecur[:]),
                             start=True, stop=True)
            p1p2 = ps_loop.tile([1, 2 * R], f32, tag="p1p2")
            nc.tensor.matmul(out=p1p2[:], lhsT=r(onesD[:]), rhs=r(pr[:]),
                             start=True, stop=True)

            # --- update: U' = (1+e)*U + cand ---
            V_sb = work.tile([D, R], f32, tag="V")
            VE.scalar_tensor_tensor(out=V_sb[:], in0=psbce[:, 0:R],
                                    scalar=1.0, in1=c(Ucur[:, 0:R]),
                                    op0=OP.add, op1=OP.mult)
            VE.tensor_tensor(out=Unext[:, 0:R], in0=V_sb[:], in1=cand[:],
                             op=OP.add)

            # --- norm bookkeeping: ss' = (1+e)((1+e)ss + 2 p1) + p2 ---
            m1 = work.tile([1, R], f32, tag="m1")
            VE.scalar_tensor_tensor(out=m1[:], in0=c(ecur[:, 0:R]), scalar=1.0,
                                    in1=ss, op0=OP.add, op1=OP.mult)
            m2 = work.tile([1, R], f32, tag="m2")
            VE.scalar_tensor_tensor(out=m2[:], in0=p1p2[:, 0:R], scalar=2.0,
                                    in1=m1[:], op0=OP.mult, op1=OP.add)
            m3 = work.tile([1, R], f32, tag="m3")
            VE.scalar_tensor_tensor(out=m3[:], in0=c(ecur[:, 0:R]), scalar=1.0,
                                    in1=m2[:], op0=OP.add, op1=OP.mult)
            ss_new = work.tile([1, R], f32, tag="ssn")
            VE.tensor_tensor(out=ss_new[:], in0=m3[:], in1=p1p2[:, R:2 * R],
                             op=OP.add)
            lnss = work.tile([1, R], f32, tag="lnss")
            AC.activation(out=lnss[:], in_=ss_new[:], func=AF.Ln)
            AC.activation(out=rcur[:, 0:R], in_=lnss[:], func=AF.Exp,
                          scale=-0.5)

            if (t + 1) % RESCALE == 0:
                # exact renormalization: U <- U'*rho'; ss/rho/n -> exactly 1
                psbcr = ps_loop.tile([D, RP], f32, tag="psbce", name="psbcr")
                nc.tensor.matmul(out=psbcr[:], lhsT=r(ones1[:]), rhs=r(rcur[:]),
                                 start=True, stop=True)
                Un2 = Utiles[(ucur + 2) % 3]
                VE.tensor_tensor(out=Un2[:, 0:R], in0=psbcr[:, 0:R],
                                 in1=c(Unext[:, 0:R]), op=OP.mult)
                ucur = (ucur + 2) % 3
                ss = c(onesR)
                rho = None
                n_pad = None
            else:
                G.tensor_tensor(out=ncur[:, 0:R], in0=ss_new[:],
                                in1=c(rcur[:, 0:R]), op=OP.mult)
                ucur = (ucur + 1) % 3
                ss = ss_new[:]
                rho = c(rcur[:, 0:R])
                n_pad = ncur[:]

        # ---- output: mem = U (last step rescaled) or U*rho ----
        Ufin = Utiles[ucur]
        if n_steps % RESCALE == 0:
            nc.sync.dma_start(out=out_d, in_=c(Ufin[:, 0:R]))
        else:
            psbcr = ps_loop.tile([D, RP], f32, tag="psbce", name="psbcr_f")
            rfin = rho_t[(n_steps - 1) % 2]
            nc.tensor.matmul(out=psbcr[:], lhsT=r(ones1[:]), rhs=r(rfin[:]),
                             start=True, stop=True)
            memT = work.tile([D, R], f32, tag="memT")
            nc.vector.tensor_tensor(out=memT[:], in0=psbcr[:, 0:R],
                                    in1=c(Ufin[:, 0:R]), op=OP.mult)
            nc.sync.dma_start(out=out_d, in_=memT[:])

    _strip_redundant_self_waits(nc)
    return nc


def _stage_inputs(tokens, emb, keys, mult, Uw, Vw, Ww, prelu_a):
    """Host-side sharding/layout prep. Returns (in_maps, flags)."""
    tokens = np.asarray(tokens)
    emb = np.ascontiguousarray(np.asarray(emb, dtype=np.float32))
    keys = np.asarray(keys, dtype=np.float32)
    mult = np.asarray(mult, dtype=np.float32)
    a = float(np.asarray(prelu_a).reshape(-1)[0])
    a_is_one = (a == 1.0)
    mult_is_ones = bool(np.all(mult == 1.0))

    CW = 686 if not mult_is_ones else 586
    consts = np.zeros((128, CW), np.float32)
    consts[0:D, 0:M] = keys.T
    consts[0:D, 20:120] = np.asarray(Uw, np.float32).T        # lhsT for Uw@mem
    consts[0:D, 120:220] = np.asarray(Ww, np.float32).T
    consts[0:D, 220:320] = np.asarray(Vw, np.float32).T
    consts[0:128, 320:324] = np.kron(np.eye(4, dtype=np.float32),
                                     np.ones((32, 1), np.float32))
    consts[0:D, 324:325] = 1.0                                # onesD
    consts[0:1, 325:425] = 1.0                                # ones1
    consts[0:1, 425:585] = 1.0                                # onesR
    if not mult_is_ones:
        consts[0:128, 585:685] = np.tile(mult, (4, 1))

    in_maps = []
    for c in range(NCORES):
        tc_ = tokens[c * BL:(c + 1) * BL]                     # [8, S, L]
        # sentence-major rows with t-major sentence order: row j = 8t+b
        tokflat = np.ascontiguousarray(tc_.transpose(1, 0, 2)).reshape(NS, L)
        # tok_staged[p, col] = token of sentence 4*col + p//32, word p%32
        tok_staged = np.ascontiguousarray(
            tokflat.reshape(2 * S, 4, L).transpose(1, 2, 0)).reshape(128, 2 * S)
        in_maps.append({"tok": np.ascontiguousarray(tok_staged, np.int32),
                        "emb": emb, "consts": consts})
    return in_maps, a_is_one, mult_is_ones, a


def kernel(tokens, emb, keys, mult, Uw, Vw, Ww, prelu_a, _trace=False):
    from concourse.bass_utils import run_bass_kernel_spmd

    in_maps, a_is_one, mult_is_ones, a = _stage_inputs(
        tokens, emb, keys, mult, Uw, Vw, Ww, prelu_a)

    fat = os.environ.get("K_FAT", "1") == "1"
    gtt = os.environ.get("K_GTT", "1") == "1"
    key = (a_is_one, mult_is_ones, a, fat, gtt)
    if key not in _prog_cache:
        _prog_cache[key] = _build_program(a_is_one, mult_is_ones, a,
                                          fat_gather=fat, gpsimd_tt=gtt)
    nc = _prog_cache[key]

    res = run_bass_kernel_spmd(nc, in_maps, list(range(NCORES)), trace=_trace)
    out = np.empty((B, M, D), dtype=np.float32)
    for c in range(NCORES):
        memT = res.results[c]["memT"]                          # [D, R]
        out[c * BL:(c + 1) * BL] = memT.reshape(D, BL, M).transpose(1, 2, 0)
    kernel._last_results = res
    return out


# revision 23
# speedup vs baseline: 1.2973x; 1.0142x over previous
"""EntNet Trainium2 kernel (8-core data-parallel over batch), v3.

Reference computation (B=64, S=128, L=32, D=100, M=20, V=50000):
  sents = (emb[tokens] * mult).sum(axis=2)            # [B,S,D]
  mem0 = broadcast(keys)                              # [B,M,D]
  per step t: gate = sigmoid(s.mem + s.keys); cand = prelu(mem@Uw.T + keys@Vw.T + s@Ww.T)
              mem = normalize(mem + cand*gate, axis=D)

Kernel strategy per core (8 batches/core), layout [D, rows] with D on
partitions:
  - Embedding gather via gpsimd indirect DMA (one index per partition, 16
    consecutive single-column gathers into one block tile so the SWDGE can
    coalesce descriptor generation), reduced words->sentences with PE
    block-ones matmuls into D-major sents [100, 1024] (t-major cols 8t+b).
  - TWO independent recurrence chains of 4 batches each (rows R2=80),
    interleaved at a half-step offset, so each chain's serial dependency
    cycle is hidden behind the other chain's engine work.
  - Per chain, scale-free form: mem = rho*U, U unnormalized, n = 1/rho:
        l  = rho*(s.U) + s.keys         e = exp(-l)
        U' = (1+e)*U + candf            candf = Uw@U + (Vk + Ws_t)*n
        ss = ||U'||^2 (square+colsum), rho' = exp(-.5 ln ss),
        n' = exp(+.5 ln ss)  (computed in parallel with rho' on ACT)
    Exact renormalization every RESCALE steps resets rho/n to exactly 1.
  - The two gate reductions ride ONE [100,256] f32r colsum matmul per chain
    (U*s | keys*s | zero pad); keys*s and Vk+Ws are materialized once per
    8-step block as strided block ops, not per step. The e/n row-broadcasts
    share one [1,512] f32r matmul per chain per step.
  - f32r (single-pass PE) with moving dim padded to >=256 for the recurrence
    matmuls; fp32 elsewhere.
"""

import os

import numpy as np

B, S, L, D, M, V = 64, 128, 32, 100, 20, 50000
NCORES = 8
BL = B // NCORES            # 8 batches per core
NCH = 2                     # recurrence chains per core
BC = BL // NCH              # 4 batches per chain
R = BL * M                  # 160 rows per core
R2 = BC * M                 # 80 rows per chain
NS = BL * S
NTOK = BL * S * L
NGATH = NTOK // 128         # 256 gather index columns
NBLK = 16                   # sentence blocks (64 sent cols = 8 steps)
RESCALE = 8
RP = 256                    # padded moving dim for f32r matmuls

_prog_cache = {}

_ENGINE_SEM = {"PE": "PE_", "DVE": "DVE_", "Activation": "Activation_",
               "Pool": "Pool_", "SP": "SP_"}


def _strip_redundant_self_waits(nc):
    """Legalize sync waits: walrus rejects >1 sync wait on most instruction
    structs. For any instruction carrying several, hoist all but one onto
    preceding single-wait NoOps on the same engine queue (in-order dispatch
    keeps semantics). The instruction keeps its OWN-engine wait if it has one
    (that wait guards an engine-pipelining RAW hazard and must gate execution,
    not just dispatch).
    """
    import concourse.mybir as mybir
    for fn in nc.m.functions:
        for blk in fn.blocks:
            i = 0
            while i < len(blk.instructions):
                inst = blk.instructions[i]
                si = inst.sync_info() if callable(inst.sync_info) else inst.sync_info
                if (si is not None and si.on_wait and len(si.on_wait) > 1
                        and inst.engine is not None):
                    waits = list(si.on_wait)
                    pref = _ENGINE_SEM.get(inst.engine.name)
                    keep_idx = None
                    for j, w in enumerate(waits):
                        if pref and w.ant_name.startswith(pref):
                            keep_idx = j
                            break
                    kept = [waits.pop(keep_idx)] if keep_idx is not None else []
                    noops = []
                    for w in waits:
                        nop = mybir.InstNoOp(
                            name=nc.get_next_instruction_name(), ins=[], outs=[])
                        nop.engine = inst.engine
                        nop.sync_info = mybir.SyncInfo(on_wait=[w], on_update=[])
                        nc.register_instruction(nop, overwrite=True)
                        noops.append(nop)
                    inst.sync_info = mybir.SyncInfo(
                        on_wait=kept, on_update=list(si.on_update))
                    blk.instructions[i:i] = noops
                    i += len(noops)
                i += 1


def _build_program(a_is_one: bool, mult_is_ones: bool, alpha: float,
                   n_steps: int = S):
    import concourse.bass as bass
    import concourse.tile as tile
    from concourse import mybir
    from contextlib import ExitStack

    f32 = mybir.dt.float32
    f32r = mybir.dt.float32r
    i32 = mybir.dt.int32
    AF = mybir.ActivationFunctionType
    OP = mybir.AluOpType

    nc = bass.Bass(trn_type="TRN2")

    CW = 686 if not mult_is_ones else 586
    tok_d = nc.dram_tensor("tok", [128, NGATH], i32, kind="ExternalInput").ap()
    emb_d = nc.dram_tensor("emb", [V, D], f32, kind="ExternalInput").ap()
    consts_d = nc.dram_tensor("consts", [128, CW], f32r, kind="ExternalInput").ap()
    out_d = nc.dram_tensor("memT", [D, R], f32, kind="ExternalOutput").ap()

    def c(ap):
        # plain-f32 view of an f32r tensor for non-matmul readers
        return ap.bitcast(f32)

    def bcast_mid(ap_2d, n_mid):
        # [P, k] -> [P, n_mid, k] with stride-0 middle dim
        return bass.AP(ap_2d.tensor, ap_2d.offset,
                       [list(ap_2d.ap[0]), [0, n_mid], list(ap_2d.ap[1])])

    def bcast_last(ap_2d, n_last):
        # [P, k] -> [P, k, n_last] with stride-0 last dim
        return bass.AP(ap_2d.tensor, ap_2d.offset,
                       [list(ap_2d.ap[0]), list(ap_2d.ap[1]), [0, n_last]])

    def bcast_mid2(ap_2d, n1, n2):
        # [P, k] -> [P, n1, n2, k] with stride-0 dims 1 and 2
        return bass.AP(ap_2d.tensor, ap_2d.offset,
                       [list(ap_2d.ap[0]), [0, n1], [0, n2], list(ap_2d.ap[1])])

    with tile.TileContext(nc) as tc, ExitStack() as ctx:
        const = ctx.enter_context(tc.tile_pool(name="const", bufs=1))
        gpool = ctx.enter_context(tc.tile_pool(name="gath", bufs=4))
        work = ctx.enter_context(tc.tile_pool(name="work", bufs=2))
        ps_setup = ctx.enter_context(tc.tile_pool(name="ps_setup", bufs=2, space="PSUM"))
        ps_loop = ctx.enter_context(tc.tile_pool(name="ps_loop", bufs=1, space="PSUM"))

        # ---- constants ----
        tok_sb = const.tile([128, NGATH], i32)
        nc.gpsimd.dma_start(out=tok_sb[:], in_=tok_d)
        consts = const.tile([128, CW], f32r)
        nc.sync.dma_start(out=consts[:], in_=consts_d)
        keysT = consts[0:D, 0:M]
        UwT = consts[0:D, 20:120]
        WwT = consts[0:D, 120:220]
        VwT = consts[0:D, 220:320]
        blk = consts[0:128, 320:324]
        onesD = consts[0:D, 324:325]
        ones1 = consts[0:1, 325:425]
        if not mult_is_ones:
            multT = consts[0:128, 586:686]
        zcolD = consts[0:D, CW - 1:CW]
        zcol1 = consts[0:1, CW - 1:CW]

        # ---- Vk = Vw @ keys^T ----
        ps_vk = ps_setup.tile([D, M], f32, tag="pssent", bufs=3, name="ps_vk")
        nc.tensor.matmul(out=ps_vk[:], lhsT=c(VwT[:]), rhs=c(keysT[:]),
                         start=True, stop=True)
        Vk = const.tile([D, M], f32)
        nc.vector.tensor_copy(out=Vk[:], in_=ps_vk[:])

        # ---- gather machinery ----
        sents_b = [const.tile([D, 64], f32, name=f"sents_b{w}")
                   for w in range(NBLK)]
        Ws_b = [const.tile([D, 64], f32, name=f"ws_b{w}") for w in range(NBLK)]
        gtiles = {}
        ps_blks = {}

        def emit_gather(w):
            g = gpool.tile([128, 16, D], f32, tag="g", name=f"g{w}")
            for gi in range(16):
                nc.gpsimd.indirect_dma_start(
                    out=g[:, gi, :],
                    out_offset=None,
                    in_=emb_d,
                    in_offset=bass.IndirectOffsetOnAxis(
                        ap=tok_sb[:, w * 16 + gi:w * 16 + gi + 1], axis=0),
                )
            gtiles[w] = g

        def emit_reduce(w, quarter):
            if quarter == 0:
                ps_blks[w] = ps_setup.tile([D, 64], f32, tag="pssent", bufs=3,
                                           name=f"ps_blk{w}")
            ps_blk = ps_blks[w]
            for gi in range(4 * quarter, 4 * quarter + 4):
                gc = gtiles[w][:, gi, :]
                if not mult_is_ones:
                    gm = gpool.tile([128, D], f32, tag="gm", name=f"gm{w}_{gi}")
                    nc.vector.tensor_tensor(out=gm[:], in0=gc, in1=c(multT[:]),
                                            op=OP.mult)
                    gc = gm[:]
                nc.tensor.matmul(out=ps_blk[:, 4 * gi:4 * gi + 4],
                                 lhsT=gc, rhs=c(blk[:]), start=True, stop=True)

        def emit_finish(w):
            nc.vector.tensor_copy(out=sents_b[w][:], in_=ps_blks[w][:])
            ps_ws = ps_setup.tile([D, 64], f32, tag="pssent", bufs=3,
                                  name=f"ps_ws{w}")
            nc.tensor.matmul(out=ps_ws[:], lhsT=c(WwT[:]), rhs=sents_b[w][:],
                             start=True, stop=True)
            nc.vector.tensor_copy(out=Ws_b[w][:], in_=ps_ws[:])

        # ---- fixed per-chain state tiles ----
        U_t = [[const.tile([D, RP], f32r, name=f"U{ci}_{i}") for i in range(3)]
               for ci in range(NCH)]
        # ksB step-slice layout [100, 256]: [0:80]=U*s (per step),
        # [80:160]=keys*s (per block), [160:256]=zero pad
        ksB = [[const.tile([D, 8, RP], f32r, name=f"ksB{ci}_{i}")
                for i in range(2)] for ci in range(NCH)]
        vwB = [[const.tile([D, 8, R2], f32, name=f"vwB{ci}_{i}")
                for i in range(2)] for ci in range(NCH)]
        # en layout [1, 512]: [0:80]=n, [256:336]=e, rest zero (bcen rhs)
        en_t = [[const.tile([1, 2 * RP], f32r, name=f"en{ci}_{i}")
                 for i in range(2)] for ci in range(NCH)]
        rho_t = [[const.tile([1, RP], f32r, name=f"rho{ci}_{i}")
                  for i in range(2)] for ci in range(NCH)]
        for ci in range(NCH):
            for i in range(3):
                nc.vector.tensor_copy(out=U_t[ci][i][:, R2:RP],
                                      in_=bcast_last(zcolD, RP - R2))
            for i in range(2):
                nc.vector.tensor_copy(
                    out=ksB[ci][i][:, :, 2 * R2:RP],
                    in_=bass.AP(zcolD.tensor, zcolD.offset,
                                [list(zcolD.ap[0]), [0, 8], [0, RP - 2 * R2]]))
                nc.vector.tensor_copy(out=en_t[ci][i][:, 0:RP],
                                      in_=bcast_last(zcol1, RP))
                nc.vector.tensor_copy(out=en_t[ci][i][:, RP + R2:2 * RP],
                                      in_=bcast_last(zcol1, RP - R2))
                nc.vector.tensor_copy(out=rho_t[ci][i][:, R2:RP],
                                      in_=bcast_last(zcol1, RP - R2))
            nc.vector.tensor_copy(
                out=U_t[ci][0][:, 0:R2].rearrange("d (b m) -> d b m", m=M),
                in_=bcast_mid(c(keysT), BC))

        VE, GE, AC = nc.vector, nc.gpsimd, nc.scalar

        def emit_ksvw_block(ci, w, on_pool):
            """keys*s and Vk+Ws for all 8 steps of block w, chain ci."""
            E = GE if on_pool else VE
            kd = ksB[ci][w % 2][:, :, R2:2 * R2].rearrange(
                "p a (b m) -> p a b m", m=M)
            sb = sents_b[w]
            s_ap = bass.AP(sb.tensor, sb[:, 4 * ci:].offset,
                           [list(sb.ap[0]), [8, 8], [1, BC], [0, M]])
            E.tensor_tensor(out=kd, in0=bcast_mid2(c(keysT), 8, BC),
                            in1=s_ap, op=OP.mult)
            vd = vwB[ci][w % 2][:].rearrange("p a (b m) -> p a b m", m=M)
            wb = Ws_b[w]
            ws_ap = bass.AP(wb.tensor, wb[:, 4 * ci:].offset,
                            [list(wb.ap[0]), [8, 8], [1, BC], [0, M]])
            E.tensor_tensor(out=vd, in0=bcast_mid2(Vk[:], 8, BC),
                            in1=ws_ap, op=OP.add)

        # per-chain python state
        ucur = [0] * NCH
        rho = [None] * NCH
        has_n = [False] * NCH
        psA_ps = [None] * NCH
        bcen_ps = [None] * NCH

        def front(ci, t):
            """gate: psA matmul, mgt write, psmg matmul, l, e."""
            w, c8, k = t // 8, t % 8, t % 2
            Ucur = U_t[ci][ucur[ci]]
            psA_ps[ci] = ps_loop.tile([D, RP], f32, tag="psA",
                                      name=f"psA{ci}_{t}")
            nc.tensor.matmul(out=psA_ps[ci][:], lhsT=UwT[:], rhs=Ucur[:],
                             start=True, stop=True)
            sb = sents_b[w]
            s_sl = bass.AP(sb.tensor, sb[:, 8 * c8 + 4 * ci:].offset,
                           [list(sb.ap[0]), [1, BC], [0, M]])
            VE.tensor_tensor(
                out=ksB[ci][w % 2][:, c8, 0:R2].rearrange(
                    "d (b m) -> d b m", m=M),
                in0=c(Ucur[:, 0:R2]).rearrange("d (b m) -> d b m", m=M),
                in1=s_sl, op=OP.mult)
            psmg = ps_loop.tile([1, RP], f32, tag="psmg",
                                name=f"psmg{ci}_{t}")
            nc.tensor.matmul(out=psmg[:], lhsT=onesD[:],
                             rhs=ksB[ci][w % 2][:, c8, :], start=True, stop=True)
            l1 = work.tile([1, R2], f32, tag=f"l1_{ci}", name=f"l1{ci}_{t}")
            if rho[ci] is not None:
                VE.tensor_tensor(out=l1[:], in0=psmg[:, 0:R2], in1=rho[ci],
                                 op=OP.mult)
            else:
                VE.tensor_copy(out=l1[:], in_=psmg[:, 0:R2])
            l_sb = work.tile([1, R2], f32, tag=f"l_{ci}", name=f"l{ci}_{t}")
            VE.tensor_tensor(out=l_sb[:], in0=psmg[:, R2:2 * R2], in1=l1[:],
                             op=OP.add)
            AC.activation(out=en_t[ci][k][:, RP:RP + R2], in_=l_sb[:],
                          func=AF.Exp, scale=-1.0)

        def back(ci, t, pool_free):
            """update: bcen, candf, V, U2, norm chain."""
            w, c8, k = t // 8, t % 8, t % 2
            Ucur = U_t[ci][ucur[ci]]
            Unext = U_t[ci][(ucur[ci] + 1) % 3]
            bcen_ps[ci] = ps_loop.tile([D, 2 * RP], f32, tag="bcen", bufs=2,
                                       name=f"bcen{ci}_{t}")
            nc.tensor.matmul(out=bcen_ps[ci][:], lhsT=ones1[:],
                             rhs=en_t[ci][k][:], start=True, stop=True)
            vw_sl = vwB[ci][w % 2][:, c8, :]
            candf = work.tile([D, R2], f32, tag=f"candf_{ci}",
                              name=f"candf{ci}_{t}")
            if has_n[ci]:
                c1 = work.tile([D, R2], f32, tag=f"c1_{ci}", name=f"c1{ci}_{t}")
                VE.tensor_tensor(out=c1[:], in0=vw_sl,
                                 in1=bcen_ps[ci][:, 0:R2], op=OP.mult)
                VE.tensor_tensor(out=candf[:], in0=psA_ps[ci][:, 0:R2],
                                 in1=c1[:], op=OP.add)
            else:
                VE.tensor_tensor(out=candf[:], in0=psA_ps[ci][:, 0:R2],
                                 in1=vw_sl, op=OP.add)
            cand = candf
            if not a_is_one:
                candp = work.tile([D, R2], f32, tag=f"candp_{ci}",
                                  name=f"candp{ci}_{t}")
                AC.activation(out=candp[:], in_=candf[:], func=AF.Prelu,
                              alpha=float(alpha))
                cand = candp
            V_sb = work.tile([D, R2], f32, tag=f"V_{ci}", name=f"V{ci}_{t}")
            VE.scalar_tensor_tensor(out=V_sb[:], in0=bcen_ps[ci][:, RP:RP + R2],
                                    scalar=1.0, in1=c(Ucur[:, 0:R2]),
                                    op0=OP.add, op1=OP.mult)
            VE.tensor_tensor(out=Unext[:, 0:R2], in0=V_sb[:], in1=cand[:],
                             op=OP.add)
            # norm chain: sq -> colsum -> ln -> {rho, n}
            sq = work.tile([D, R2], f32, tag=f"sq_{ci}", name=f"sq{ci}_{t}")
            SE = GE if (ci == 0 or pool_free) else VE
            SE.tensor_tensor(out=sq[:], in0=c(Unext[:, 0:R2]),
                             in1=c(Unext[:, 0:R2]), op=OP.mult)
            psss = ps_loop.tile([1, R2], f32, tag="psss",
                                name=f"psss{ci}_{t}")
            nc.tensor.matmul(out=psss[:], lhsT=c(onesD[:]), rhs=sq[:],
                             start=True, stop=True)
            lnss = work.tile([1, R2], f32, tag=f"lnss_{ci}", name=f"lnss{ci}_{t}")
            AC.activation(out=lnss[:], in_=psss[:], func=AF.Ln)
            kn = (t + 1) % 2
            AC.activation(out=rho_t[ci][kn][:, 0:R2], in_=lnss[:],
                          func=AF.Exp, scale=-0.5)
            if (t + 1) % RESCALE == 0:
                psbcr = ps_loop.tile([D, RP], f32, tag="bcen", bufs=2,
                                     name=f"psbcr{ci}_{t}")
                nc.tensor.matmul(out=psbcr[:], lhsT=ones1[:],
                                 rhs=rho_t[ci][kn][:], start=True, stop=True)
                Un2 = U_t[ci][(ucur[ci] + 2) % 3]
                VE.tensor_tensor(out=Un2[:, 0:R2], in0=psbcr[:, 0:R2],
                                 in1=c(Unext[:, 0:R2]), op=OP.mult)
                ucur[ci] = (ucur[ci] + 2) % 3
                rho[ci] = None
                has_n[ci] = False
            else:
                AC.activation(out=en_t[ci][kn][:, 0:R2], in_=lnss[:],
                              func=AF.Exp, scale=0.5)
                ucur[ci] = (ucur[ci] + 1) % 3
                rho[ci] = c(rho_t[ci][kn][:, 0:R2])
                has_n[ci] = True

        # ---- prologue: blocks 0 and 1, ks/vw for block 0 ----
        for w in (0, 1):
            emit_gather(w)
            for q in range(4):
                emit_reduce(w, q)
            emit_finish(w)
        for ci in range(NCH):
            emit_ksvw_block(ci, 0, on_pool=False)

        # ---- main loop: chains interleaved at half-step offset ----
        for t in range(n_steps):
            w, c8 = t // 8, t % 8
            pool_free = w + 2 >= NBLK
            if c8 == 2 and w + 2 < NBLK:
                emit_gather(w + 2)
            if c8 >= 4 and w + 2 < NBLK:
                emit_reduce(w + 2, c8 - 4)
                if c8 == 7:
                    emit_finish(w + 2)
            if c8 == 5 and w + 1 < NBLK:
                emit_ksvw_block(0, w + 1, on_pool=pool_free)
            if c8 == 6 and w + 1 < NBLK:
                emit_ksvw_block(1, w + 1, on_pool=pool_free)

            front(0, t)
            if t > 0:
                back(1, t - 1, w + 2 >= NBLK)
            back(0, t, pool_free)
            front(1, t)
        back(1, n_steps - 1, True)

        # ---- output ----
        for ci in range(NCH):
            Ufin = U_t[ci][ucur[ci]]
            if n_steps % RESCALE == 0:
                nc.sync.dma_start(out=out_d[:, R2 * ci:R2 * (ci + 1)],
                                  in_=c(Ufin[:, 0:R2]))
            else:
                psbcr = ps_loop.tile([D, RP], f32, tag="bcen", bufs=2,
                                     name=f"psbcrf{ci}")
                rfin = rho_t[ci][n_steps % 2]
                nc.tensor.matmul(out=psbcr[:], lhsT=ones1[:], rhs=rfin[:],
                                 start=True, stop=True)
                memT = work.tile([D, R2], f32, tag=f"memT_{ci}",
                                 name=f"memT{ci}")
                nc.vector.tensor_tensor(out=memT[:], in0=psbcr[:, 0:R2],
                                        in1=c(Ufin[:, 0:R2]), op=OP.mult)
                nc.sync.dma_start(out=out_d[:, R2 * ci:R2 * (ci + 1)],
                                  in_=memT[:])

    _strip_redundant_self_waits(nc)
    return nc


def _stage_inputs(tokens, emb, keys, mult, Uw, Vw, Ww, prelu_a):
    """Host-side sharding/layout prep. Returns (in_maps, flags)."""
    tokens = np.asarray(tokens)
    emb = np.ascontiguousarray(np.asarray(emb, dtype=np.float32))
    keys = np.asarray(keys, dtype=np.float32)
    mult = np.asarray(mult, dtype=np.float32)
    a = float(np.asarray(prelu_a).reshape(-1)[0])
    a_is_one = (a == 1.0)
    mult_is_ones = bool(np.all(mult == 1.0))

    CW = 686 if not mult_is_ones else 586
    consts = np.zeros((128, CW), np.float32)
    consts[0:D, 0:M] = keys.T
    consts[0:D, 20:120] = np.asarray(Uw, np.float32).T        # lhsT for Uw@mem
    consts[0:D, 120:220] = np.asarray(Ww, np.float32).T
    consts[0:D, 220:320] = np.asarray(Vw, np.float32).T
    consts[0:128, 320:324] = np.kron(np.eye(4, dtype=np.float32),
                                     np.ones((32, 1), np.float32))
    consts[0:D, 324:325] = 1.0                                # onesD
    consts[0:1, 325:425] = 1.0                                # ones1
    if not mult_is_ones:
        consts[0:128, 586:686] = np.tile(mult, (4, 1))

    in_maps = []
    for cr in range(NCORES):
        tc_ = tokens[cr * BL:(cr + 1) * BL]                   # [8, S, L]
        # sentence-major rows with t-major sentence order: row j = 8t+b
        tokflat = np.ascontiguousarray(tc_.transpose(1, 0, 2)).reshape(NS, L)
        # tok_staged[p, col] = token of sentence 4*col + p//32, word p%32
        tok_staged = np.ascontiguousarray(
            tokflat.reshape(2 * S, 4, L).transpose(1, 2, 0)).reshape(128, 2 * S)
        in_maps.append({"tok": np.ascontiguousarray(tok_staged, np.int32),
                        "emb": emb, "consts": consts})
    return in_maps, a_is_one, mult_is_ones, a


def kernel(tokens, emb, keys, mult, Uw, Vw, Ww, prelu_a, _trace=False):
    from concourse.bass_utils import run_bass_kernel_spmd

    in_maps, a_is_one, mult_is_ones, a = _stage_inputs(
        tokens, emb, keys, mult, Uw, Vw, Ww, prelu_a)

    key = (a_is_one, mult_is_ones, a)
    if key not in _prog_cache:
        _prog_cache[key] = _build_program(a_is_one, mult_is_ones, a)
    nc = _prog_cache[key]

    res = run_bass_kernel_spmd(nc, in_maps, list(range(NCORES)), trace=_trace)
    out = np.empty((B, M, D), dtype=np.float32)
    for cr in range(NCORES):
        memT = res.results[cr]["memT"]                         # [D, R]
        out[cr * BL:(cr + 1) * BL] = memT.reshape(D, BL, M).transpose(1, 2, 0)
    kernel._last_results = res
    return out


# revision 24
# speedup vs baseline: 1.3437x; 1.0358x over previous
"""EntNet Trainium2 kernel (8-core data-parallel over batch), v3.

Reference computation (B=64, S=128, L=32, D=100, M=20, V=50000):
  sents = (emb[tokens] * mult).sum(axis=2)            # [B,S,D]
  mem0 = broadcast(keys)                              # [B,M,D]
  per step t: gate = sigmoid(s.mem + s.keys); cand = prelu(mem@Uw.T + keys@Vw.T + s@Ww.T)
              mem = normalize(mem + cand*gate, axis=D)

Kernel strategy per core (8 batches/core), layout [D, rows] with D on
partitions:
  - Embedding gather via gpsimd indirect DMA (one index per partition, 16
    consecutive single-column gathers into one block tile so the SWDGE can
    coalesce descriptor generation), reduced words->sentences with PE
    block-ones matmuls into D-major sents [100, 1024] (t-major cols 8t+b).
  - TWO independent recurrence chains of 4 batches each (rows R2=80),
    interleaved at a half-step offset, so each chain's serial dependency
    cycle is hidden behind the other chain's engine work.
  - Per chain, scale-free form: mem = rho*U, U unnormalized, n = 1/rho:
        l  = rho*(s.U) + s.keys         e = exp(-l)
        U' = (1+e)*U + candf            candf = Uw@U + (Vk + Ws_t)*n
        ss = ||U'||^2 (square+colsum), rho' = exp(-.5 ln ss),
        n' = exp(+.5 ln ss)  (computed in parallel with rho' on ACT)
    Exact renormalization every RESCALE steps resets rho/n to exactly 1.
  - The two gate reductions ride ONE [100,256] f32r colsum matmul per chain
    (U*s | keys*s | zero pad); keys*s and Vk+Ws are materialized once per
    8-step block as strided block ops, not per step. The e/n row-broadcasts
    share one [1,512] f32r matmul per chain per step.
  - f32r (single-pass PE) with moving dim padded to >=256 for the recurrence
    matmuls; fp32 elsewhere.
"""

import os

import numpy as np

B, S, L, D, M, V = 64, 128, 32, 100, 20, 50000
NCORES = 8
BL = B // NCORES            # 8 batches per core
NCH = 2                     # recurrence chains per core
BC = BL // NCH              # 4 batches per chain
R = BL * M                  # 160 rows per core
R2 = BC * M                 # 80 rows per chain
NS = BL * S
NTOK = BL * S * L
NGATH = NTOK // 128         # 256 gather index columns
NBLK = 16                   # sentence blocks (64 sent cols = 8 steps)
RESCALE = 8
RP = 256                    # padded moving dim for f32r matmuls

_prog_cache = {}

_ENGINE_SEM = {"PE": "PE_", "DVE": "DVE_", "Activation": "Activation_",
               "Pool": "Pool_", "SP": "SP_"}


def _strip_redundant_self_waits(nc):
    """Legalize sync waits: walrus rejects >1 sync wait on most instruction
    structs. For any instruction carrying several, hoist all but one onto
    preceding single-wait NoOps on the same engine queue (in-order dispatch
    keeps semantics). The instruction keeps its OWN-engine wait if it has one
    (that wait guards an engine-pipelining RAW hazard and must gate execution,
    not just dispatch).
    """
    import concourse.mybir as mybir
    for fn in nc.m.functions:
        for blk in fn.blocks:
            i = 0
            while i < len(blk.instructions):
                inst = blk.instructions[i]
                si = inst.sync_info() if callable(inst.sync_info) else inst.sync_info
                if (si is not None and si.on_wait and len(si.on_wait) > 1
                        and inst.engine is not None):
                    waits = list(si.on_wait)
                    pref = _ENGINE_SEM.get(inst.engine.name)
                    keep_idx = None
                    for j, w in enumerate(waits):
                        if pref and w.ant_name.startswith(pref):
                            keep_idx = j
                            break
                    kept = [waits.pop(keep_idx)] if keep_idx is not None else []
                    noops = []
                    for w in waits:
                        nop = mybir.InstNoOp(
                            name=nc.get_next_instruction_name(), ins=[], outs=[])
                        nop.engine = inst.engine
                        nop.sync_info = mybir.SyncInfo(on_wait=[w], on_update=[])
                        nc.register_instruction(nop, overwrite=True)
                        noops.append(nop)
                    inst.sync_info = mybir.SyncInfo(
                        on_wait=kept, on_update=list(si.on_update))
                    blk.instructions[i:i] = noops
                    i += len(noops)
                i += 1


def _build_program(a_is_one: bool, mult_is_ones: bool, alpha: float,
                   n_steps: int = S):
    import concourse.bass as bass
    import concourse.tile as tile
    from concourse import mybir
    from contextlib import ExitStack

    f32 = mybir.dt.float32
    f32r = mybir.dt.float32r
    i32 = mybir.dt.int32
    AF = mybir.ActivationFunctionType
    OP = mybir.AluOpType

    nc = bass.Bass(trn_type="TRN2")

    CW = 686 if not mult_is_ones else 586
    tok_d = nc.dram_tensor("tok", [128, NGATH], i32, kind="ExternalInput").ap()
    emb_d = nc.dram_tensor("emb", [V, D], f32, kind="ExternalInput").ap()
    consts_d = nc.dram_tensor("consts", [128, CW], f32r, kind="ExternalInput").ap()
    out_d = nc.dram_tensor("memT", [D, R], f32, kind="ExternalOutput").ap()

    def c(ap):
        # plain-f32 view of an f32r tensor for non-matmul readers
        return ap.bitcast(f32)

    def bcast_mid(ap_2d, n_mid):
        # [P, k] -> [P, n_mid, k] with stride-0 middle dim
        return bass.AP(ap_2d.tensor, ap_2d.offset,
                       [list(ap_2d.ap[0]), [0, n_mid], list(ap_2d.ap[1])])

    def bcast_last(ap_2d, n_last):
        # [P, k] -> [P, k, n_last] with stride-0 last dim
        return bass.AP(ap_2d.tensor, ap_2d.offset,
                       [list(ap_2d.ap[0]), list(ap_2d.ap[1]), [0, n_last]])

    def bcast_mid2(ap_2d, n1, n2):
        # [P, k] -> [P, n1, n2, k] with stride-0 dims 1 and 2
        return bass.AP(ap_2d.tensor, ap_2d.offset,
                       [list(ap_2d.ap[0]), [0, n1], [0, n2], list(ap_2d.ap[1])])

    with tile.TileContext(nc) as tc, ExitStack() as ctx:
        const = ctx.enter_context(tc.tile_pool(name="const", bufs=1))
        gpool = ctx.enter_context(tc.tile_pool(name="gath", bufs=4))
        work = ctx.enter_context(tc.tile_pool(name="work", bufs=2))
        ps_setup = ctx.enter_context(tc.tile_pool(name="ps_setup", bufs=2, space="PSUM"))
        ps_loop = ctx.enter_context(tc.tile_pool(name="ps_loop", bufs=1, space="PSUM"))

        # ---- constants ----
        tok_sb = const.tile([128, NGATH], i32)
        nc.gpsimd.dma_start(out=tok_sb[:], in_=tok_d)
        consts = const.tile([128, CW], f32r)
        nc.sync.dma_start(out=consts[:], in_=consts_d)
        keysT = consts[0:D, 0:M]
        UwT = consts[0:D, 20:120]
        WwT = consts[0:D, 120:220]
        VwT = consts[0:D, 220:320]
        blk = consts[0:128, 320:324]
        onesD = consts[0:D, 324:325]
        ones1 = consts[0:1, 325:425]
        if not mult_is_ones:
            multT = consts[0:128, 586:686]
        zcolD = consts[0:D, CW - 1:CW]
        zcol1 = consts[0:1, CW - 1:CW]

        # ---- Vk = Vw @ keys^T ----
        ps_vk = ps_setup.tile([D, M], f32, tag="pssent", bufs=3, name="ps_vk")
        nc.tensor.matmul(out=ps_vk[:], lhsT=c(VwT[:]), rhs=c(keysT[:]),
                         start=True, stop=True)
        Vk = const.tile([D, M], f32)
        nc.vector.tensor_copy(out=Vk[:], in_=ps_vk[:])

        # ---- gather machinery ----
        sents_b = [const.tile([D, 64], f32, name=f"sents_b{w}")
                   for w in range(NBLK)]
        Ws_b = [const.tile([D, 64], f32, name=f"ws_b{w}") for w in range(NBLK)]
        gtiles = {}
        ps_blks = {}

        def emit_gather(w):
            g = gpool.tile([128, 16, D], f32, tag="g", name=f"g{w}")
            for gi in range(16):
                nc.gpsimd.indirect_dma_start(
                    out=g[:, gi, :],
                    out_offset=None,
                    in_=emb_d,
                    in_offset=bass.IndirectOffsetOnAxis(
                        ap=tok_sb[:, w * 16 + gi:w * 16 + gi + 1], axis=0),
                )
            gtiles[w] = g

        def emit_reduce(w, quarter):
            if quarter == 0:
                ps_blks[w] = ps_setup.tile([D, 64], f32, tag="pssent", bufs=3,
                                           name=f"ps_blk{w}")
            ps_blk = ps_blks[w]
            for gi in range(4 * quarter, 4 * quarter + 4):
                gc = gtiles[w][:, gi, :]
                if not mult_is_ones:
                    gm = gpool.tile([128, D], f32, tag="gm", name=f"gm{w}_{gi}")
                    nc.vector.tensor_tensor(out=gm[:], in0=gc, in1=c(multT[:]),
                                            op=OP.mult)
                    gc = gm[:]
                nc.tensor.matmul(out=ps_blk[:, 4 * gi:4 * gi + 4],
                                 lhsT=gc, rhs=c(blk[:]), start=True, stop=True)

        def emit_finish(w):
            nc.vector.tensor_copy(out=sents_b[w][:], in_=ps_blks[w][:])
            ps_ws = ps_setup.tile([D, 64], f32, tag="pssent", bufs=3,
                                  name=f"ps_ws{w}")
            nc.tensor.matmul(out=ps_ws[:], lhsT=c(WwT[:]), rhs=sents_b[w][:],
                             start=True, stop=True)
            nc.vector.tensor_copy(out=Ws_b[w][:], in_=ps_ws[:])

        # ---- fixed per-chain state tiles ----
        U_t = [[const.tile([D, RP], f32r, name=f"U{ci}_{i}") for i in range(3)]
               for ci in range(NCH)]
        # ksB step-slice layout [100, 256]: [0:80]=U*s (per step),
        # [80:160]=keys*s (per block), [160:256]=zero pad
        ksB = [[const.tile([D, 8, RP], f32r, name=f"ksB{ci}_{i}")
                for i in range(2)] for ci in range(NCH)]
        vwB = [[const.tile([D, 8, R2], f32, name=f"vwB{ci}_{i}")
                for i in range(2)] for ci in range(NCH)]
        # en layout [1, 512]: [0:80]=n, [256:336]=e, rest zero (bcen rhs)
        en_t = [[const.tile([1, 2 * RP], f32r, name=f"en{ci}_{i}")
                 for i in range(2)] for ci in range(NCH)]
        rho_t = [[const.tile([1, RP], f32r, name=f"rho{ci}_{i}")
                  for i in range(2)] for ci in range(NCH)]
        sq_t = [[const.tile([D, RP], f32r, name=f"sq{ci}_{i}")
                 for i in range(2)] for ci in range(NCH)]
        for ci in range(NCH):
            for i in range(3):
                nc.vector.tensor_copy(out=U_t[ci][i][:, R2:RP],
                                      in_=bcast_last(zcolD, RP - R2))
            for i in range(2):
                nc.vector.tensor_copy(out=sq_t[ci][i][:, R2:RP],
                                      in_=bcast_last(zcolD, RP - R2))
            for i in range(2):
                nc.vector.tensor_copy(
                    out=ksB[ci][i][:, :, 2 * R2:RP],
                    in_=bass.AP(zcolD.tensor, zcolD.offset,
                                [list(zcolD.ap[0]), [0, 8], [0, RP - 2 * R2]]))
                nc.vector.tensor_copy(out=en_t[ci][i][:, 0:RP],
                                      in_=bcast_last(zcol1, RP))
                nc.vector.tensor_copy(out=en_t[ci][i][:, RP + R2:2 * RP],
                                      in_=bcast_last(zcol1, RP - R2))
                nc.vector.tensor_copy(out=rho_t[ci][i][:, R2:RP],
                                      in_=bcast_last(zcol1, RP - R2))
            nc.vector.tensor_copy(
                out=U_t[ci][0][:, 0:R2].rearrange("d (b m) -> d b m", m=M),
                in_=bcast_mid(c(keysT), BC))

        VE, GE, AC = nc.vector, nc.gpsimd, nc.scalar

        def emit_ksvw_block(ci, w, on_pool):
            """keys*s and Vk+Ws for all 8 steps of block w, chain ci."""
            E = GE if on_pool else VE
            kd = ksB[ci][w % 2][:, :, R2:2 * R2].rearrange(
                "p a (b m) -> p a b m", m=M)
            sb = sents_b[w]
            s_ap = bass.AP(sb.tensor, sb[:, 4 * ci:].offset,
                           [list(sb.ap[0]), [8, 8], [1, BC], [0, M]])
            E.tensor_tensor(out=kd, in0=bcast_mid2(c(keysT), 8, BC),
                            in1=s_ap, op=OP.mult)
            vd = vwB[ci][w % 2][:].rearrange("p a (b m) -> p a b m", m=M)
            wb = Ws_b[w]
            ws_ap = bass.AP(wb.tensor, wb[:, 4 * ci:].offset,
                            [list(wb.ap[0]), [8, 8], [1, BC], [0, M]])
            E.tensor_tensor(out=vd, in0=bcast_mid2(Vk[:], 8, BC),
                            in1=ws_ap, op=OP.add)

        # per-chain python state
        ucur = [0] * NCH
        rho = [None] * NCH
        has_n = [False] * NCH
        psA_ps = [None] * NCH
        bcen_ps = [None] * NCH

        def front(ci, t):
            """gate: psA matmul, mgt write, psmg matmul, l, e."""
            w, c8, k = t // 8, t % 8, t % 2
            Ucur = U_t[ci][ucur[ci]]
            psA_ps[ci] = ps_loop.tile([D, RP], f32, tag="psA",
                                      name=f"psA{ci}_{t}")
            nc.tensor.matmul(out=psA_ps[ci][:], lhsT=UwT[:], rhs=Ucur[:],
                             start=True, stop=True)
            sb = sents_b[w]
            s_sl = bass.AP(sb.tensor, sb[:, 8 * c8 + 4 * ci:].offset,
                           [list(sb.ap[0]), [1, BC], [0, M]])
            VE.tensor_tensor(
                out=ksB[ci][w % 2][:, c8, 0:R2].rearrange(
                    "d (b m) -> d b m", m=M),
                in0=c(Ucur[:, 0:R2]).rearrange("d (b m) -> d b m", m=M),
                in1=s_sl, op=OP.mult)
            psmg = ps_loop.tile([1, RP], f32, tag="psmg",
                                name=f"psmg{ci}_{t}")
            nc.tensor.matmul(out=psmg[:], lhsT=onesD[:],
                             rhs=ksB[ci][w % 2][:, c8, :], start=True, stop=True)
            l1 = work.tile([1, R2], f32, tag=f"l1_{ci}", name=f"l1{ci}_{t}")
            if rho[ci] is not None:
                VE.tensor_tensor(out=l1[:], in0=psmg[:, 0:R2], in1=rho[ci],
                                 op=OP.mult)
            else:
                VE.tensor_copy(out=l1[:], in_=psmg[:, 0:R2])
            l_sb = work.tile([1, R2], f32, tag=f"l_{ci}", name=f"l{ci}_{t}")
            VE.tensor_tensor(out=l_sb[:], in0=psmg[:, R2:2 * R2], in1=l1[:],
                             op=OP.add)
            AC.activation(out=en_t[ci][k][:, RP:RP + R2], in_=l_sb[:],
                          func=AF.Exp, scale=-1.0)

        def back_a(ci, t):
            """update: bcen, candf, V, U2."""
            w, c8, k = t // 8, t % 8, t % 2
            Ucur = U_t[ci][ucur[ci]]
            Unext = U_t[ci][(ucur[ci] + 1) % 3]
            bcen_ps[ci] = ps_loop.tile([D, 336], f32, tag="bcen", bufs=2,
                                       name=f"bcen{ci}_{t}")
            nc.tensor.matmul(out=bcen_ps[ci][:], lhsT=ones1[:],
                             rhs=en_t[ci][k][:, 0:336], start=True, stop=True)
            vw_sl = vwB[ci][w % 2][:, c8, :]
            candf = work.tile([D, R2], f32, tag=f"candf_{ci}",
                              name=f"candf{ci}_{t}")
            if has_n[ci]:
                c1 = work.tile([D, R2], f32, tag=f"c1_{ci}", name=f"c1{ci}_{t}")
                VE.tensor_tensor(out=c1[:], in0=vw_sl,
                                 in1=bcen_ps[ci][:, 0:R2], op=OP.mult)
                VE.tensor_tensor(out=candf[:], in0=psA_ps[ci][:, 0:R2],
                                 in1=c1[:], op=OP.add)
            else:
                VE.tensor_tensor(out=candf[:], in0=psA_ps[ci][:, 0:R2],
                                 in1=vw_sl, op=OP.add)
            cand = candf
            if not a_is_one:
                candp = work.tile([D, R2], f32, tag=f"candp_{ci}",
                                  name=f"candp{ci}_{t}")
                AC.activation(out=candp[:], in_=candf[:], func=AF.Prelu,
                              alpha=float(alpha))
                cand = candp
            V_sb = work.tile([D, R2], f32, tag=f"V_{ci}", name=f"V{ci}_{t}")
            VE.scalar_tensor_tensor(out=V_sb[:], in0=bcen_ps[ci][:, RP:RP + R2],
                                    scalar=1.0, in1=c(Ucur[:, 0:R2]),
                                    op0=OP.add, op1=OP.mult)
            VE.tensor_tensor(out=Unext[:, 0:R2], in0=V_sb[:], in1=cand[:],
                             op=OP.add)

        def back_b(ci, t, pool_free):
            """norm chain: sq -> colsum -> ln -> {rho, n}; rescale."""
            k = t % 2
            Unext = U_t[ci][(ucur[ci] + 1) % 3]
            sq = sq_t[ci][k]
            SE = GE if (ci == 0 or pool_free) else VE
            SE.tensor_tensor(out=sq[:, 0:R2], in0=c(Unext[:, 0:R2]),
                             in1=c(Unext[:, 0:R2]), op=OP.mult)
            psss = ps_loop.tile([1, RP], f32, tag="psss",
                                name=f"psss{ci}_{t}")
            nc.tensor.matmul(out=psss[:], lhsT=onesD[:], rhs=sq[:],
                             start=True, stop=True)
            lnss = work.tile([1, R2], f32, tag=f"lnss_{ci}", name=f"lnss{ci}_{t}")
            AC.activation(out=lnss[:], in_=psss[:, 0:R2], func=AF.Ln)
            kn = (t + 1) % 2
            AC.activation(out=rho_t[ci][kn][:, 0:R2], in_=lnss[:],
                          func=AF.Exp, scale=-0.5)
            if (t + 1) % RESCALE == 0:
                psbcr = ps_loop.tile([D, RP], f32, tag="bcen", bufs=2,
                                     name=f"psbcr{ci}_{t}")
                nc.tensor.matmul(out=psbcr[:], lhsT=ones1[:],
                                 rhs=rho_t[ci][kn][:], start=True, stop=True)
                Un2 = U_t[ci][(ucur[ci] + 2) % 3]
                VE.tensor_tensor(out=Un2[:, 0:R2], in0=psbcr[:, 0:R2],
                                 in1=c(Unext[:, 0:R2]), op=OP.mult)
                ucur[ci] = (ucur[ci] + 2) % 3
                rho[ci] = None
                has_n[ci] = False
            else:
                AC.activation(out=en_t[ci][kn][:, 0:R2], in_=lnss[:],
                              func=AF.Exp, scale=0.5)
                ucur[ci] = (ucur[ci] + 1) % 3
                rho[ci] = c(rho_t[ci][kn][:, 0:R2])
                has_n[ci] = True

        # ---- prologue: blocks 0 and 1, ks/vw for block 0 ----
        for w in (0, 1):
            emit_gather(w)
            for q in range(4):
                emit_reduce(w, q)
            emit_finish(w)
        for ci in range(NCH):
            emit_ksvw_block(ci, 0, on_pool=False)

        # ---- main loop: chains interleaved at half-step offset ----
        for t in range(n_steps):
            w, c8 = t // 8, t % 8
            pool_free = w + 2 >= NBLK
            if c8 == 2 and w + 2 < NBLK:
                emit_gather(w + 2)
            if c8 >= 4 and w + 2 < NBLK:
                emit_reduce(w + 2, c8 - 4)
                if c8 == 7:
                    emit_finish(w + 2)
            if c8 == 5 and w + 1 < NBLK:
                emit_ksvw_block(0, w + 1, on_pool=pool_free)
            if c8 == 6 and w + 1 < NBLK:
                emit_ksvw_block(1, w + 1, on_pool=pool_free)

            front(0, t)
            if t > 0:
                back_b(1, t - 1, w + 2 >= NBLK)
            back_a(0, t)
            front(1, t)
            back_b(0, t, pool_free)
            back_a(1, t)
        back_b(1, n_steps - 1, True)

        # ---- output ----
        for ci in range(NCH):
            Ufin = U_t[ci][ucur[ci]]
            if n_steps % RESCALE == 0:
                nc.sync.dma_start(out=out_d[:, R2 * ci:R2 * (ci + 1)],
                                  in_=c(Ufin[:, 0:R2]))
            else:
                psbcr = ps_loop.tile([D, RP], f32, tag="bcen", bufs=2,
                                     name=f"psbcrf{ci}")
                rfin = rho_t[ci][n_steps % 2]
                nc.tensor.matmul(out=psbcr[:], lhsT=ones1[:], rhs=rfin[:],
                                 start=True, stop=True)
                memT = work.tile([D, R2], f32, tag=f"memT_{ci}",
                                 name=f"memT{ci}")
                nc.vector.tensor_tensor(out=memT[:], in0=psbcr[:, 0:R2],
                                        in1=c(Ufin[:, 0:R2]), op=OP.mult)
                nc.sync.dma_start(out=out_d[:, R2 * ci:R2 * (ci + 1)],
                                  in_=memT[:])

    _strip_redundant_self_waits(nc)
    return nc


def _stage_inputs(tokens, emb, keys, mult, Uw, Vw, Ww, prelu_a):
    """Host-side sharding/layout prep. Returns (in_maps, flags)."""
    tokens = np.asarray(tokens)
    emb = np.ascontiguousarray(np.asarray(emb, dtype=np.float32))
    keys = np.asarray(keys, dtype=np.float32)
    mult = np.asarray(mult, dtype=np.float32)
    a = float(np.asarray(prelu_a).reshape(-1)[0])
    a_is_one = (a == 1.0)
    mult_is_ones = bool(np.all(mult == 1.0))

    CW = 686 if not mult_is_ones else 586
    consts = np.zeros((128, CW), np.float32)
    consts[0:D, 0:M] = keys.T
    consts[0:D, 20:120] = np.asarray(Uw, np.float32).T        # lhsT for Uw@mem
    consts[0:D, 120:220] = np.asarray(Ww, np.float32).T
    consts[0:D, 220:320] = np.asarray(Vw, np.float32).T
    consts[0:128, 320:324] = np.kron(np.eye(4, dtype=np.float32),
                                     np.ones((32, 1), np.float32))
    consts[0:D, 324:325] = 1.0                                # onesD
    consts[0:1, 325:425] = 1.0                                # ones1
    if not mult_is_ones:
        consts[0:128, 586:686] = np.tile(mult, (4, 1))

    in_maps = []
    for cr in range(NCORES):
        tc_ = tokens[cr * BL:(cr + 1) * BL]                   # [8, S, L]
        # sentence-major rows with t-major sentence order: row j = 8t+b
        tokflat = np.ascontiguousarray(tc_.transpose(1, 0, 2)).reshape(NS, L)
        # tok_staged[p, col] = token of sentence 4*col + p//32, word p%32
        tok_staged = np.ascontiguousarray(
            tokflat.reshape(2 * S, 4, L).transpose(1, 2, 0)).reshape(128, 2 * S)
        in_maps.append({"tok": np.ascontiguousarray(tok_staged, np.int32),
                        "emb": emb, "consts": consts})
    return in_maps, a_is_one, mult_is_ones, a


def kernel(tokens, emb, keys, mult, Uw, Vw, Ww, prelu_a, _trace=False):
    from concourse.bass_utils import run_bass_kernel_spmd

    in_maps, a_is_one, mult_is_ones, a = _stage_inputs(
        tokens, emb, keys, mult, Uw, Vw, Ww, prelu_a)

    key = (a_is_one, mult_is_ones, a)
    if key not in _prog_cache:
        _prog_cache[key] = _build_program(a_is_one, mult_is_ones, a)
    nc = _prog_cache[key]

    res = run_bass_kernel_spmd(nc, in_maps, list(range(NCORES)), trace=_trace)
    out = np.empty((B, M, D), dtype=np.float32)
    for cr in range(NCORES):
        memT = res.results[cr]["memT"]                         # [D, R]
        out[cr * BL:(cr + 1) * BL] = memT.reshape(D, BL, M).transpose(1, 2, 0)
    kernel._last_results = res
    return out
